# revision 1
# baseline (speedup 1.0000x reference)
"""APPNP GNN kernel for 8 Trainium2 NeuronCores (Bass/Tile).

Node-sharded (data/graph parallel):
  - Host: relabel nodes per dst-core, degree-sorted grouping, padded CSR slot
    layout, int16 wrapped gather indices (all index-derived preprocessing).
  - Device per core: MLP (channel-major matmuls, fused bias+relu in ACT
    drains), then K=10 propagation hops:
      AllGather hs -> HBM table; per src-slice dma_gather of 256B rows into
      SBUF; grouped tensor_reduce segment-sum on DVE; per-slice partials
      combined via small static permutation gathers; DVE epilogue
      h = a*h0 + (1-a)*norm_in*(sum_t m_t); hs = h*norm_out.
  - Per-edge norm factorizes: norm_out folded before gather, norm_in after.
"""
import math
import numpy as np

import concourse.bacc as bacc
import concourse.mybir as mybir
from concourse import tile
from concourse.masks import make_identity
from concourse.bass_utils import run_bass_kernel_spmd

F32 = mybir.dt.float32
I16 = mybir.dt.int16
DEBUG_STAGE = None
DEBUG_NOAG = False
DEBUG_LEVEL = 6  # 1=AG 2=+gather 3=+reduce 4=+mtDMA 5=+combine 6=full

NCORES = 8
NSLICES = 4
C = 48          # output channels
TW = 64         # table row width (floats) -> 256B rows
ALPHA = 0.1
KHOPS = 10
FIN = 512
H1 = 256
H2 = 256


class Config:
    def __init__(self, n_nodes):
        self.N = n_nodes
        assert n_nodes % NCORES == 0
        self.real_per = n_nodes // NCORES                 # real nodes per core
        self.PER = ((self.real_per + 128) // 128) * 128   # >=1 dummy slot
        self.NG = self.PER // 128                         # node groups per core
        self.ROWS = NCORES * self.PER                     # hfull rows
        self.SLICE_ROWS = 2 * self.PER                    # rows per gather slice
        assert self.SLICE_ROWS <= 32767
        self.MLP_CHUNK = 512                              # nodes per MLP chunk
        self.GCOLS_MAX = 64                               # gather-call column cap


class Meta:
    pass


def preprocess(edge_index, cfg):
    N, PER, NG = cfg.N, cfg.PER, cfg.NG
    real_per = cfg.real_per
    src = np.asarray(edge_index[0]).astype(np.int64)
    dst = np.asarray(edge_index[1]).astype(np.int64)

    deg_out = np.bincount(src, minlength=N).astype(np.float32)
    deg_in = np.bincount(dst, minlength=N).astype(np.float32)
    no_full = 1.0 / np.sqrt(np.maximum(deg_out, 1.0))
    ni_full = 1.0 / np.sqrt(np.maximum(deg_in, 1.0))

    core_of = np.minimum(np.arange(N) // real_per, NCORES - 1)

    # slice of an edge = src core pair (row-order independent)
    ecore = core_of[dst]
    eslice = core_of[src] // 2
    dloc = dst - ecore * real_per
    d_ct = np.zeros((NCORES, NSLICES, real_per), dtype=np.int64)
    for c in range(NCORES):
        for t in range(NSLICES):
            m = (ecore == c) & (eslice == t)
            d_ct[c, t] = np.bincount(dloc[m], minlength=real_per)

    pos_t = np.zeros((NCORES, NSLICES, real_per), dtype=np.int64)
    order_t = np.zeros((NCORES, NSLICES, real_per), dtype=np.int64)
    for c in range(NCORES):
        for t in range(NSLICES):
            o = np.argsort(d_ct[c, t], kind="stable")
            order_t[c, t] = o
            pos_t[c, t, o] = np.arange(real_per)

    # common order := slice-0 order (kills slice-0's combine permutation)
    POS = np.zeros(N, dtype=np.int64)
    com_order = np.zeros((NCORES, real_per), dtype=np.int64)
    for c in range(NCORES):
        ids = np.arange(c * real_per, (c + 1) * real_per)
        com_order[c] = ids[order_t[c, 0]]
        POS[com_order[c]] = np.arange(real_per)
    ROW = core_of * PER + POS
    srow = ROW[src]
    assert np.array_equal(eslice, srow // cfg.SLICE_ROWS)

    meta = Meta()
    meta.cfg = cfg
    meta.com_order = com_order

    # group max slice-degree, common schedule across cores (dummies at end)
    D = np.zeros((NSLICES, NG), dtype=np.int64)
    for t in range(NSLICES):
        for c in range(NCORES):
            dd = d_ct[c, t][order_t[c, t]]
            dd = np.concatenate([dd, np.zeros(PER - real_per, dtype=np.int64)])
            D[t] = np.maximum(D[t], dd.reshape(NG, 128).max(axis=1))
    meta.D = D
    CUM = np.zeros((NSLICES, NG + 1), dtype=np.int64)
    for t in range(NSLICES):
        CUM[t, 1:] = np.cumsum(D[t])
    meta.CUM = CUM
    meta.COLS = CUM[:, -1].copy()

    # chunk packing per slice
    chunks = []
    for t in range(NSLICES):
        g0 = 0
        while g0 < NG:
            g1, cols = g0, 0
            while g1 < NG and cols + D[t, g1] <= cfg.GCOLS_MAX:
                cols += int(D[t, g1])
                g1 += 1
            if g1 == g0:
                raise RuntimeError(f"group D={D[t, g0]} exceeds GCOLS_MAX")
            chunks.append((t, g0, g1, int(CUM[t, g0]), cols))
            g0 = g1
    meta.chunks = chunks

    PAD = real_per                       # slice-local dummy row (zero forever)
    slice_off = np.zeros(NSLICES + 1, dtype=np.int64)
    slice_off[1:] = np.cumsum(128 * meta.COLS)
    gidx_flat = np.zeros((NCORES, int(slice_off[-1])), dtype=np.int64)
    for c in range(NCORES):
        for t in range(NSLICES):
            flat = np.full(int(128 * meta.COLS[t]), PAD, dtype=np.int64)
            eidx = np.where((ecore == c) & (eslice == t))[0]
            q = pos_t[c, t, dloc[eidx]]
            o = np.argsort(q, kind="stable")
            eo, qs = eidx[o], q[o]
            starts = np.searchsorted(qs, np.arange(real_per))
            r = np.arange(len(qs)) - starts[qs]
            col = CUM[t, qs // 128] + r
            flat[col * 128 + (qs % 128)] = srow[eo] - t * cfg.SLICE_ROWS
            gidx_flat[c, slice_off[t]:slice_off[t + 1]] = flat

    def wrap(flat):
        n = len(flat)
        assert n % 16 == 0
        w = flat.reshape(n // 16, 16).T.astype(np.int16)
        return np.tile(w, (8, 1))

    gidx = []
    for c in range(NCORES):
        blocks = []
        for (t, g0, g1, c0, cols) in chunks:
            if cols == 0:
                continue
            lo = int(slice_off[t] + 128 * c0)
            blocks.append(wrap(gidx_flat[c, lo:lo + 128 * cols]))
        gidx.append(np.concatenate(blocks, axis=1))
    meta.gidx = np.stack(gidx)
    meta.GW = meta.gidx.shape[2]

    cidx = np.zeros((NCORES, NSLICES, PER), dtype=np.int64)
    for c in range(NCORES):
        loc = com_order[c] - c * real_per
        for t in range(NSLICES):
            cidx[c, t, :real_per] = pos_t[c, t, loc]
            cidx[c, t, real_per:] = np.arange(real_per, PER)
    meta.cidx = np.stack([
        np.stack([wrap(cidx[c, t]) for t in range(NSLICES)]) for c in range(NCORES)
    ])  # [NCORES, NSLICES, 128, PER/16]

    no_c = np.zeros((NCORES, PER), dtype=np.float32)
    nip_c = np.zeros((NCORES, PER), dtype=np.float32)
    am_c = np.zeros((NCORES, PER), dtype=np.float32)
    for c in range(NCORES):
        no_c[c, :real_per] = no_full[com_order[c]]
        nip_c[c, :real_per] = (1.0 - ALPHA) * ni_full[com_order[c]]
        am_c[c, :real_per] = ALPHA
    meta.no_c, meta.nip_c, meta.am_c = no_c, nip_c, am_c
    return meta


def build_nc(meta):
    cfg = meta.cfg
    PER, NG, ROWS = cfg.PER, cfg.NG, cfg.ROWS
    nc = bacc.Bacc("TRN2", target_bir_lowering=False, debug=False, num_devices=NCORES,
                   num_swdge_queues=4)

    xT = nc.declare_dram_parameter("xT", [FIN, PER], F32, isOutput=False)
    W0 = nc.declare_dram_parameter("W0", [FIN, H1], F32, isOutput=False)
    b0 = nc.declare_dram_parameter("b0", [H1], F32, isOutput=False)
    W1 = nc.declare_dram_parameter("W1", [H1, H2], F32, isOutput=False)
    b1 = nc.declare_dram_parameter("b1", [H2], F32, isOutput=False)
    W2 = nc.declare_dram_parameter("W2", [H2, C], F32, isOutput=False)
    b2 = nc.declare_dram_parameter("b2", [C], F32, isOutput=False)
    no_d = nc.declare_dram_parameter("no", [PER], F32, isOutput=False)
    nip_d = nc.declare_dram_parameter("nip", [PER], F32, isOutput=False)
    am_d = nc.declare_dram_parameter("am", [PER], F32, isOutput=False)
    gidx_d = nc.declare_dram_parameter("gidx", [128, meta.GW], I16, isOutput=False)
    ident_d = nc.declare_dram_parameter("ident", [128, 128], F32, isOutput=False)
    cidx_d = nc.declare_dram_parameter("cidx", [NSLICES, 128, PER // 16], I16, isOutput=False)
    out_d = nc.declare_dram_parameter("out", [PER, C], F32, isOutput=True)

    bounce = nc.dram_tensor("bounce", [PER, TW], F32)
    hfull = [nc.dram_tensor(f"hfull{i}", [ROWS, TW], F32, addr_space="Shared")
             for i in range(2)]
    Mt = [nc.dram_tensor(f"Mt{t}", [PER, TW], F32) for t in range(NSLICES)]
    rg = [list(range(NCORES))]

    with tile.TileContext(nc) as tc:
        with (
            tc.tile_pool(name="const", bufs=1) as constp,
            tc.tile_pool(name="state", bufs=1) as sp,
        ):
            ident = constp.tile([128, 128], F32)
            nc.sync.dma_start(ident[:, :], ident_d.ap())
            no_sb = constp.tile([128, NG, 1], F32)
            nip_sb = constp.tile([128, NG, 1], F32)
            am_sb = constp.tile([128, NG, 1], F32)
            nc.sync.dma_start(no_sb[:, :, 0], no_d.ap().rearrange("(g p) -> p g", p=128))
            nc.sync.dma_start(nip_sb[:, :, 0], nip_d.ap().rearrange("(g p) -> p g", p=128))
            nc.sync.dma_start(am_sb[:, :, 0], am_d.ap().rearrange("(g p) -> p g", p=128))
            cidx_sb = [constp.tile([128, PER // 16], I16, tag=f"cidx{t}", name=f"cidx{t}")
                       for t in range(NSLICES)]
            for t in range(NSLICES):
                nc.sync.dma_start(cidx_sb[t][:, :], cidx_d.ap()[t])

            h0a = sp.tile([128, NG, C], F32)
            zpad = constp.tile([128, NG, TW - C], F32)
            nc.vector.memset(zpad[:, :, :], 0.0)
            nc.sync.dma_start(
                bounce.ap()[:, C:TW].rearrange("(g p) c -> p g c", p=128),
                zpad[:, :, :])
            for t in range(NSLICES):
                nc.sync.dma_start(
                    Mt[t].ap()[:, C:TW].rearrange("(g p) c -> p g c", p=128),
                    zpad[:, :, :])

            # ---------------- MLP ----------------
            with (
                tc.tile_pool(name="mlpw", bufs=1) as wp,
                tc.tile_pool(name="mlpx", bufs=2) as xp,
                tc.tile_pool(name="mlph", bufs=2) as hp,
                tc.tile_pool(name="mlppsum", bufs=2, space="PSUM") as pp,
                tc.tile_pool(name="tpsum", bufs=2, space="PSUM") as tp,
            ):
                w0_sb = [wp.tile([128, H1], F32, tag=f"w0_{k}", name=f"w0_{k}") for k in range(FIN // 128)]
                for k in range(FIN // 128):
                    nc.sync.dma_start(w0_sb[k][:, :], W0.ap()[k * 128:(k + 1) * 128, :])
                w1_sb = [wp.tile([128, H2], F32, tag=f"w1_{k}", name=f"w1_{k}") for k in range(H1 // 128)]
                for k in range(H1 // 128):
                    nc.sync.dma_start(w1_sb[k][:, :], W1.ap()[k * 128:(k + 1) * 128, :])
                w2_sb = [wp.tile([128, C], F32, tag=f"w2_{k}", name=f"w2_{k}") for k in range(H2 // 128)]
                for k in range(H2 // 128):
                    nc.sync.dma_start(w2_sb[k][:, :], W2.ap()[k * 128:(k + 1) * 128, :])
                b0_sb = wp.tile([128, 2], F32, tag="b0")
                b1_sb = wp.tile([128, 2], F32, tag="b1")
                b2_sb = wp.tile([128, 1], F32, tag="b2")
                nc.sync.dma_start(b0_sb[:, :], b0.ap().rearrange("(m p) -> p m", p=128))
                nc.sync.dma_start(b1_sb[:, :], b1.ap().rearrange("(m p) -> p m", p=128))
                nc.sync.dma_start(b2_sb[0:C, 0], b2.ap())

                xTv = xT.ap().rearrange("(k p) n -> k p n", p=128)
                nchunks = math.ceil(PER / cfg.MLP_CHUNK)
                for ci in range(nchunks):
                    n0 = ci * cfg.MLP_CHUNK
                    nn = min(cfg.MLP_CHUNK, PER - n0)
                    xk = [xp.tile([128, cfg.MLP_CHUNK], F32, tag="xk", name="xk") for _ in range(4)]
                    for k in range(4):
                        nc.sync.dma_start(xk[k][:, 0:nn], xTv[k][:, n0:n0 + nn])
                    h1t = [hp.tile([128, cfg.MLP_CHUNK], F32, tag="h1", name="h1") for _ in range(2)]
                    for m in range(2):
                        ps = pp.tile([128, cfg.MLP_CHUNK], F32, tag="ps1")
                        for k in range(4):
                            nc.tensor.matmul(ps[:, 0:nn],
                                             w0_sb[k][:, m * 128:(m + 1) * 128],
                                             xk[k][:, 0:nn],
                                             start=(k == 0), stop=(k == 3))
                        nc.scalar.activation(h1t[m][:, 0:nn], ps[:, 0:nn],
                                             mybir.ActivationFunctionType.Relu,
                                             bias=b0_sb[:, m:m + 1])
                    h2t = [hp.tile([128, cfg.MLP_CHUNK], F32, tag="h2", name="h2") for _ in range(2)]
                    for m in range(2):
                        ps = pp.tile([128, cfg.MLP_CHUNK], F32, tag="ps2")
                        for k in range(2):
                            nc.tensor.matmul(ps[:, 0:nn],
                                             w1_sb[k][:, m * 128:(m + 1) * 128],
                                             h1t[k][:, 0:nn],
                                             start=(k == 0), stop=(k == 1))
                        nc.scalar.activation(h2t[m][:, 0:nn], ps[:, 0:nn],
                                             mybir.ActivationFunctionType.Relu,
                                             bias=b1_sb[:, m:m + 1])
                    h3 = hp.tile([C, cfg.MLP_CHUNK], F32, tag="h3")
                    ps3 = pp.tile([C, cfg.MLP_CHUNK], F32, tag="ps3")
                    for k in range(2):
                        nc.tensor.matmul(ps3[:, 0:nn], w2_sb[k][:, :], h2t[k][:, 0:nn],
                                         start=(k == 0), stop=(k == 1))
                    nc.scalar.activation(h3[:, 0:nn], ps3[:, 0:nn],
                                         mybir.ActivationFunctionType.Identity,
                                         bias=b2_sb[0:C, 0:1])
                    hs_stage = hp.tile([128, cfg.MLP_CHUNK // 128, C], F32, tag="hss")
                    nb = nn // 128
                    for b in range(nb):
                        j = (n0 + b * 128) // 128
                        pt = tp.tile([128, C], F32, tag="pt")
                        nc.tensor.transpose(pt[:, :], h3[:, b * 128:(b + 1) * 128],
                                            ident[0:C, 0:C])
                        nc.vector.tensor_tensor(hs_stage[:, b, :], pt[:, :],
                                                no_sb[:, j, 0:1].to_broadcast([128, C]),
                                                op=mybir.AluOpType.mult)
                        nc.vector.tensor_tensor(h0a[:, j, :], pt[:, :],
                                                am_sb[:, j, 0:1].to_broadcast([128, C]),
                                                op=mybir.AluOpType.mult)
                    nc.sync.dma_start(
                        bounce.ap()[n0:n0 + nn, 0:C].rearrange("(b p) c -> p b c", p=128),
                        hs_stage[:, 0:nb, :])

            # ---------------- propagation ----------------
            with (
                tc.tile_pool(name="gat", bufs=2) as gp,
                tc.tile_pool(name="gix", bufs=2) as ip,
                tc.tile_pool(name="mt", bufs=2) as mtp,
                tc.tile_pool(name="cg", bufs=1) as cgp,
                tc.tile_pool(name="hop", bufs=1) as hopp,
            ):
                h_sb = None
                for k in range(KHOPS):
                    hf = hfull[k % 2]
                    if not DEBUG_NOAG:
                        nc.gpsimd.collective_compute(
                            "AllGather", mybir.AluOpType.bypass, replica_groups=rg,
                            ins=[bounce.ap().opt()], outs=[hf.ap().opt()])

                    if DEBUG_LEVEL < 2:
                        continue
                    goff = 0
                    slice_chunks = {t: [ch for ch in meta.chunks if ch[0] == t]
                                    for t in range(NSLICES)}
                    mt0_keep = None
                    for t in range(NSLICES):
                        mt_sb = mtp.tile([128, NG, C], F32,
                                         tag="mt0" if t == 0 else "mt")
                        if t == 0:
                            mt0_keep = mt_sb
                        for (_, g0, g1, c0, cols) in slice_chunks[t]:
                            if cols > 0:
                                n_idx = 128 * cols
                                idx_sb = ip.tile([128, 8 * cols], I16, tag="gix")
                                nc.sync.dma_start(idx_sb[:, :],
                                                  gidx_d.ap()[:, goff:goff + 8 * cols])
                                G = gp.tile([128, cfg.GCOLS_MAX, TW], F32, tag="G")
                                nc.gpsimd.dma_gather(
                                    G[:, 0:cols, :],
                                    hf.ap()[t * cfg.SLICE_ROWS:(t + 1) * cfg.SLICE_ROWS, :],
                                    idx_sb[:, :], n_idx, n_idx, TW,
                                    single_packet=False, queue_num=t)
                                goff += 8 * cols
                            if DEBUG_LEVEL < 3:
                                continue
                            g, cc = g0, 0
                            while g < g1:
                                d = int(meta.D[t, g])
                                ge = g
                                while ge < g1 and int(meta.D[t, ge]) == d:
                                    ge += 1
                                if d == 0:
                                    nc.vector.memset(mt_sb[:, g:ge, :], 0.0)
                                else:
                                    gv = G[:, cc:cc + (ge - g) * d, 0:C].rearrange(
                                        "p (g r) c -> p g c r", r=d)
                                    nc.vector.tensor_reduce(
                                        mt_sb[:, g:ge, :], gv,
                                        axis=mybir.AxisListType.X,
                                        op=mybir.AluOpType.add)
                                cc += (ge - g) * d
                                g = ge
                        if DEBUG_LEVEL >= 4 and t > 0:
                            nc.sync.dma_start(
                                Mt[t].ap()[:, 0:C].rearrange("(g p) c -> p g c", p=128),
                                mt_sb[:, :, :])

                    if DEBUG_LEVEL < 5:
                        continue
                    msum = hopp.tile([128, NG, C], F32, tag="msum")
                    nc.vector.tensor_copy(msum[:, :, :], mt0_keep[:, :, :])
                    for t in range(1, NSLICES):
                        cg = cgp.tile([128, NG, TW], F32, tag="cg")
                        nc.gpsimd.dma_gather(
                            cg[:, :, :], Mt[t].ap()[:, :], cidx_sb[t][:, :],
                            PER, PER, TW, single_packet=False, queue_num=t)
                        nc.vector.tensor_add(msum[:, :, :], msum[:, :, :],
                                             cg[:, :, 0:C])

                    if DEBUG_LEVEL < 6:
                        continue
                    nc.vector.tensor_tensor(msum[:, :, :], msum[:, :, :],
                                            nip_sb[:, :, 0:1].to_broadcast([128, NG, C]),
                                            op=mybir.AluOpType.mult)
                    h_sb = msum
                    nc.vector.tensor_add(h_sb[:, :, :], h_sb[:, :, :], h0a[:, :, :])
                    if k < KHOPS - 1:
                        hs_sb = hopp.tile([128, NG, C], F32, tag="hs")
                        nc.vector.tensor_tensor(hs_sb[:, :, :], h_sb[:, :, :],
                                                no_sb[:, :, 0:1].to_broadcast([128, NG, C]),
                                                op=mybir.AluOpType.mult)
                        nc.sync.dma_start(
                            bounce.ap()[:, 0:C].rearrange("(g p) c -> p g c", p=128),
                            hs_sb[:, :, :])

                if h_sb is None:
                    h_sb = h0a
                nc.sync.dma_start(
                    out_d.ap().rearrange("(g p) c -> p g c", p=128), h_sb[:, :, :])

    nc.compile()
    return nc


def run_kernel(meta, inputs, nc=None, trace=False):
    cfg = meta.cfg
    if nc is None:
        nc = build_nc(meta)
    features = np.asarray(inputs["features"], dtype=np.float32)
    in_maps = []
    for c in range(NCORES):
        xTa = np.zeros((FIN, cfg.PER), dtype=np.float32)
        xTa[:, :cfg.real_per] = features[meta.com_order[c]].T
        in_maps.append({
            "xT": np.ascontiguousarray(xTa),
            "W0": np.asarray(inputs["W0"], dtype=np.float32),
            "b0": np.asarray(inputs["b0"], dtype=np.float32),
            "W1": np.asarray(inputs["W1"], dtype=np.float32),
            "b1": np.asarray(inputs["b1"], dtype=np.float32),
            "W2": np.asarray(inputs["W2"], dtype=np.float32),
            "b2": np.asarray(inputs["b2"], dtype=np.float32),
            "no": meta.no_c[c],
            "nip": meta.nip_c[c],
            "am": meta.am_c[c],
            "gidx": meta.gidx[c],
            "ident": np.eye(128, dtype=np.float32),
            "cidx": meta.cidx[c],
            "out": np.zeros((cfg.PER, C), dtype=np.float32),
        })
    res = run_bass_kernel_spmd(nc, in_maps, core_ids=list(range(NCORES)),
                               trace=trace)
    out = np.zeros((cfg.N, C), dtype=np.float32)
    for c in range(NCORES):
        oc = np.asarray(res.results[c]["out"])
        out[meta.com_order[c]] = oc[:cfg.real_per]
    return out, res


def kernel(**inputs):
    cfg = Config(100000)
    meta = preprocess(np.asarray(inputs["edge_index"]), cfg)
    out, _ = run_kernel(meta, inputs)
    return out



# revision 8
# speedup vs baseline: 2.0128x; 2.0128x over previous
"""APPNP GNN kernel for 8 Trainium2 NeuronCores (Bass/Tile).

Node-sharded (data/graph parallel):
  - Host: relabel nodes per dst-core, degree-sorted grouping, padded CSR slot
    layout, int16 wrapped gather indices (all index-derived preprocessing).
  - Device per core: MLP (channel-major matmuls, fused bias+relu in ACT
    drains), then K=10 propagation hops:
      AllGather hs -> HBM table; per src-slice dma_gather of 256B rows into
      SBUF; grouped tensor_reduce segment-sum on DVE; per-slice partials
      combined via small static permutation gathers; DVE epilogue
      h = a*h0 + (1-a)*norm_in*(sum_t m_t); hs = h*norm_out.
  - Per-edge norm factorizes: norm_out folded before gather, norm_in after.
"""
import math
import numpy as np

import concourse.bacc as bacc
import concourse.mybir as mybir
from concourse import tile
from concourse.masks import make_identity
from concourse.bass_utils import run_bass_kernel_spmd

F32 = mybir.dt.float32
I16 = mybir.dt.int16
DEBUG_STAGE = None
DEBUG_NOAG = False
DEBUG_LEVEL = 6  # 1=AG 2=+gather 3=+reduce 4=+mtDMA 5=+combine 6=full

NCORES = 8
NSLICES = 4
C = 48          # output channels
TW = 64         # table row width (floats) -> 256B rows
ALPHA = 0.1
KHOPS = 10
FIN = 512
H1 = 256
H2 = 256


class Config:
    def __init__(self, n_nodes):
        self.N = n_nodes
        assert n_nodes % NCORES == 0
        self.real_per = n_nodes // NCORES                 # real nodes per core
        self.PER = ((self.real_per + 128) // 128) * 128   # >=1 dummy slot
        self.NG = self.PER // 128                         # node groups per core
        self.ROWS = NCORES * self.PER                     # hfull rows
        self.SLICE_ROWS = 2 * self.PER                    # rows per gather slice
        assert self.SLICE_ROWS <= 32767
        self.MLP_CHUNK = 512                              # nodes per MLP chunk
        self.GCOLS_MAX = 64                               # gather-call column cap


class Meta:
    pass


def preprocess(edge_index, cfg):
    N, PER, NG = cfg.N, cfg.PER, cfg.NG
    real_per = cfg.real_per
    src = np.asarray(edge_index[0]).astype(np.int64)
    dst = np.asarray(edge_index[1]).astype(np.int64)

    deg_out = np.bincount(src, minlength=N).astype(np.float32)
    deg_in = np.bincount(dst, minlength=N).astype(np.float32)
    no_full = 1.0 / np.sqrt(np.maximum(deg_out, 1.0))
    ni_full = 1.0 / np.sqrt(np.maximum(deg_in, 1.0))

    core_of = np.minimum(np.arange(N) // real_per, NCORES - 1)

    # slice of an edge = src core pair (row-order independent)
    ecore = core_of[dst]
    eslice = core_of[src] // 2
    dloc = dst - ecore * real_per
    d_ct = np.zeros((NCORES, NSLICES, real_per), dtype=np.int64)
    for c in range(NCORES):
        for t in range(NSLICES):
            m = (ecore == c) & (eslice == t)
            d_ct[c, t] = np.bincount(dloc[m], minlength=real_per)

    pos_t = np.zeros((NCORES, NSLICES, real_per), dtype=np.int64)
    order_t = np.zeros((NCORES, NSLICES, real_per), dtype=np.int64)
    for c in range(NCORES):
        for t in range(NSLICES):
            o = np.argsort(d_ct[c, t], kind="stable")
            order_t[c, t] = o
            pos_t[c, t, o] = np.arange(real_per)

    # common order := slice-0 order (kills slice-0's combine permutation)
    POS = np.zeros(N, dtype=np.int64)
    com_order = np.zeros((NCORES, real_per), dtype=np.int64)
    for c in range(NCORES):
        ids = np.arange(c * real_per, (c + 1) * real_per)
        com_order[c] = ids[order_t[c, 0]]
        POS[com_order[c]] = np.arange(real_per)
    ROW = core_of * PER + POS
    srow = ROW[src]
    assert np.array_equal(eslice, srow // cfg.SLICE_ROWS)

    meta = Meta()
    meta.cfg = cfg
    meta.com_order = com_order

    # group max slice-degree, common schedule across cores (dummies at end)
    D = np.zeros((NSLICES, NG), dtype=np.int64)
    for t in range(NSLICES):
        for c in range(NCORES):
            dd = d_ct[c, t][order_t[c, t]]
            dd = np.concatenate([dd, np.zeros(PER - real_per, dtype=np.int64)])
            D[t] = np.maximum(D[t], dd.reshape(NG, 128).max(axis=1))
    meta.D = D
    CUM = np.zeros((NSLICES, NG + 1), dtype=np.int64)
    for t in range(NSLICES):
        CUM[t, 1:] = np.cumsum(D[t])
    meta.CUM = CUM
    meta.COLS = CUM[:, -1].copy()

    # chunk packing per slice; slice 0 last so slice-1..3 combines overlap
    # slice-0 gathers (slice 0 reduces straight into msum, no permutation)
    meta.slice_order = [1, 2, 3, 0]
    chunks = []
    for t in meta.slice_order:
        g0 = 0
        while g0 < NG:
            g1, cols = g0, 0
            while g1 < NG and cols + D[t, g1] <= cfg.GCOLS_MAX:
                cols += int(D[t, g1])
                g1 += 1
            if g1 == g0:
                raise RuntimeError(f"group D={D[t, g0]} exceeds GCOLS_MAX")
            chunks.append((t, g0, g1, int(CUM[t, g0]), cols))
            g0 = g1
    meta.chunks = chunks

    PAD = real_per                       # slice-local dummy row (zero forever)
    slice_off = np.zeros(NSLICES + 1, dtype=np.int64)
    slice_off[1:] = np.cumsum(128 * meta.COLS)
    gidx_flat = np.zeros((NCORES, int(slice_off[-1])), dtype=np.int64)
    for c in range(NCORES):
        for t in range(NSLICES):
            flat = np.full(int(128 * meta.COLS[t]), PAD, dtype=np.int64)
            eidx = np.where((ecore == c) & (eslice == t))[0]
            q = pos_t[c, t, dloc[eidx]]
            o = np.argsort(q, kind="stable")
            eo, qs = eidx[o], q[o]
            starts = np.searchsorted(qs, np.arange(real_per))
            r = np.arange(len(qs)) - starts[qs]
            col = CUM[t, qs // 128] + r
            flat[col * 128 + (qs % 128)] = srow[eo] - t * cfg.SLICE_ROWS
            gidx_flat[c, slice_off[t]:slice_off[t + 1]] = flat

    def wrap(flat):
        n = len(flat)
        assert n % 16 == 0
        w = flat.reshape(n // 16, 16).T.astype(np.int16)
        return np.tile(w, (8, 1))

    gidx = []
    for c in range(NCORES):
        blocks = []
        for (t, g0, g1, c0, cols) in chunks:
            if cols == 0:
                continue
            lo = int(slice_off[t] + 128 * c0)
            blocks.append(wrap(gidx_flat[c, lo:lo + 128 * cols]))
        gidx.append(np.concatenate(blocks, axis=1))
    meta.gidx = np.stack(gidx)
    meta.GW = meta.gidx.shape[2]

    cidx = np.zeros((NCORES, NSLICES, PER), dtype=np.int64)
    for c in range(NCORES):
        loc = com_order[c] - c * real_per
        for t in range(NSLICES):
            cidx[c, t, :real_per] = pos_t[c, t, loc]
            cidx[c, t, real_per:] = np.arange(real_per, PER)
    meta.cidx = np.stack([
        np.stack([wrap(cidx[c, t]) for t in range(NSLICES)]) for c in range(NCORES)
    ])  # [NCORES, NSLICES, 128, PER/16]

    no_c = np.zeros((NCORES, PER), dtype=np.float32)
    nip_c = np.zeros((NCORES, PER), dtype=np.float32)
    am_c = np.zeros((NCORES, PER), dtype=np.float32)
    for c in range(NCORES):
        no_c[c, :real_per] = no_full[com_order[c]]
        nip_c[c, :real_per] = (1.0 - ALPHA) * ni_full[com_order[c]]
        am_c[c, :real_per] = ALPHA
    meta.no_c, meta.nip_c, meta.am_c = no_c, nip_c, am_c
    return meta


def build_nc(meta):
    cfg = meta.cfg
    PER, NG, ROWS = cfg.PER, cfg.NG, cfg.ROWS
    nc = bacc.Bacc("TRN2", target_bir_lowering=False, debug=False, num_devices=NCORES,
                   num_swdge_queues=4)

    xT = nc.declare_dram_parameter("xT", [FIN, PER], F32, isOutput=False)
    W0 = nc.declare_dram_parameter("W0", [FIN, H1], F32, isOutput=False)
    b0 = nc.declare_dram_parameter("b0", [H1], F32, isOutput=False)
    W1 = nc.declare_dram_parameter("W1", [H1, H2], F32, isOutput=False)
    b1 = nc.declare_dram_parameter("b1", [H2], F32, isOutput=False)
    W2 = nc.declare_dram_parameter("W2", [H2, C], F32, isOutput=False)
    b2 = nc.declare_dram_parameter("b2", [C], F32, isOutput=False)
    no_d = nc.declare_dram_parameter("no", [PER], F32, isOutput=False)
    nip_d = nc.declare_dram_parameter("nip", [PER], F32, isOutput=False)
    am_d = nc.declare_dram_parameter("am", [PER], F32, isOutput=False)
    gidx_d = nc.declare_dram_parameter("gidx", [128, meta.GW], I16, isOutput=False)
    ident_d = nc.declare_dram_parameter("ident", [128, 128], F32, isOutput=False)
    cidx_d = nc.declare_dram_parameter("cidx", [NSLICES, 128, PER // 16], I16, isOutput=False)
    out_d = nc.declare_dram_parameter("out", [PER, C], F32, isOutput=True)

    bounce = nc.dram_tensor("bounce", [PER, TW], F32)
    hfull = [nc.dram_tensor(f"hfull{i}", [ROWS, TW], F32, addr_space="Shared")
             for i in range(2)]
    Mt = [nc.dram_tensor(f"Mt{t}", [PER, TW], F32) for t in range(NSLICES)]
    rg = [list(range(NCORES))]

    with tile.TileContext(nc) as tc:
        with (
            tc.tile_pool(name="const", bufs=1) as constp,
            tc.tile_pool(name="state", bufs=1) as sp,
        ):
            ident = constp.tile([128, 128], F32)
            nc.sync.dma_start(ident[:, :], ident_d.ap())
            no_sb = constp.tile([128, NG, 1], F32)
            nip_sb = constp.tile([128, NG, 1], F32)
            am_sb = constp.tile([128, NG, 1], F32)
            nc.sync.dma_start(no_sb[:, :, 0], no_d.ap().rearrange("(g p) -> p g", p=128))
            nc.sync.dma_start(nip_sb[:, :, 0], nip_d.ap().rearrange("(g p) -> p g", p=128))
            nc.sync.dma_start(am_sb[:, :, 0], am_d.ap().rearrange("(g p) -> p g", p=128))
            cidx_sb = [constp.tile([128, PER // 16], I16, tag=f"cidx{t}", name=f"cidx{t}")
                       for t in range(NSLICES)]
            for t in range(NSLICES):
                nc.sync.dma_start(cidx_sb[t][:, :], cidx_d.ap()[t])

            h0a = sp.tile([128, NG, C], F32)

            # ---------------- MLP ----------------
            with (
                tc.tile_pool(name="mlpw", bufs=1) as wp,
                tc.tile_pool(name="mlpx", bufs=2) as xp,
                tc.tile_pool(name="mlph", bufs=2) as hp,
                tc.tile_pool(name="mlppsum", bufs=2, space="PSUM") as pp,
                tc.tile_pool(name="tpsum", bufs=2, space="PSUM") as tp,
            ):
                zpad = wp.tile([128, NG, TW - C], F32, tag="zpad")
                nc.vector.memset(zpad[:, :, :], 0.0)
                nc.sync.dma_start(
                    bounce.ap()[:, C:TW].rearrange("(g p) c -> p g c", p=128),
                    zpad[:, :, :])
                for t in range(NSLICES):
                    nc.sync.dma_start(
                        Mt[t].ap()[:, C:TW].rearrange("(g p) c -> p g c", p=128),
                        zpad[:, :, :])
                w0_sb = [wp.tile([128, H1], F32, tag=f"w0_{k}", name=f"w0_{k}") for k in range(FIN // 128)]
                for k in range(FIN // 128):
                    nc.sync.dma_start(w0_sb[k][:, :], W0.ap()[k * 128:(k + 1) * 128, :])
                w1_sb = [wp.tile([128, H2], F32, tag=f"w1_{k}", name=f"w1_{k}") for k in range(H1 // 128)]
                for k in range(H1 // 128):
                    nc.sync.dma_start(w1_sb[k][:, :], W1.ap()[k * 128:(k + 1) * 128, :])
                w2_sb = [wp.tile([128, C], F32, tag=f"w2_{k}", name=f"w2_{k}") for k in range(H2 // 128)]
                for k in range(H2 // 128):
                    nc.sync.dma_start(w2_sb[k][:, :], W2.ap()[k * 128:(k + 1) * 128, :])
                b0_sb = wp.tile([128, 2], F32, tag="b0")
                b1_sb = wp.tile([128, 2], F32, tag="b1")
                b2_sb = wp.tile([128, 1], F32, tag="b2")
                nc.sync.dma_start(b0_sb[:, :], b0.ap().rearrange("(m p) -> p m", p=128))
                nc.sync.dma_start(b1_sb[:, :], b1.ap().rearrange("(m p) -> p m", p=128))
                nc.sync.dma_start(b2_sb[0:C, 0], b2.ap())

                xTv = xT.ap().rearrange("(k p) n -> k p n", p=128)
                nchunks = math.ceil(PER / cfg.MLP_CHUNK)
                for ci in range(nchunks):
                    n0 = ci * cfg.MLP_CHUNK
                    nn = min(cfg.MLP_CHUNK, PER - n0)
                    xk = [xp.tile([128, cfg.MLP_CHUNK], F32, tag="xk", name="xk") for _ in range(4)]
                    for k in range(4):
                        nc.sync.dma_start(xk[k][:, 0:nn], xTv[k][:, n0:n0 + nn])
                    h1t = [hp.tile([128, cfg.MLP_CHUNK], F32, tag="h1", name="h1") for _ in range(2)]
                    for m in range(2):
                        ps = pp.tile([128, cfg.MLP_CHUNK], F32, tag="ps1")
                        for k in range(4):
                            nc.tensor.matmul(ps[:, 0:nn],
                                             w0_sb[k][:, m * 128:(m + 1) * 128],
                                             xk[k][:, 0:nn],
                                             start=(k == 0), stop=(k == 3))
                        nc.scalar.activation(h1t[m][:, 0:nn], ps[:, 0:nn],
                                             mybir.ActivationFunctionType.Relu,
                                             bias=b0_sb[:, m:m + 1])
                    h2t = [hp.tile([128, cfg.MLP_CHUNK], F32, tag="h2", name="h2") for _ in range(2)]
                    for m in range(2):
                        ps = pp.tile([128, cfg.MLP_CHUNK], F32, tag="ps2")
                        for k in range(2):
                            nc.tensor.matmul(ps[:, 0:nn],
                                             w1_sb[k][:, m * 128:(m + 1) * 128],
                                             h1t[k][:, 0:nn],
                                             start=(k == 0), stop=(k == 1))
                        nc.scalar.activation(h2t[m][:, 0:nn], ps[:, 0:nn],
                                             mybir.ActivationFunctionType.Relu,
                                             bias=b1_sb[:, m:m + 1])
                    h3 = hp.tile([C, cfg.MLP_CHUNK], F32, tag="h3")
                    ps3 = pp.tile([C, cfg.MLP_CHUNK], F32, tag="ps3")
                    for k in range(2):
                        nc.tensor.matmul(ps3[:, 0:nn], w2_sb[k][:, :], h2t[k][:, 0:nn],
                                         start=(k == 0), stop=(k == 1))
                    nc.scalar.activation(h3[:, 0:nn], ps3[:, 0:nn],
                                         mybir.ActivationFunctionType.Identity,
                                         bias=b2_sb[0:C, 0:1])
                    hs_stage = hp.tile([128, cfg.MLP_CHUNK // 128, C], F32, tag="hss")
                    nb = nn // 128
                    for b in range(nb):
                        j = (n0 + b * 128) // 128
                        pt = tp.tile([128, C], F32, tag="pt")
                        nc.tensor.transpose(pt[:, :], h3[:, b * 128:(b + 1) * 128],
                                            ident[0:C, 0:C])
                        nc.vector.tensor_tensor(hs_stage[:, b, :], pt[:, :],
                                                no_sb[:, j, 0:1].to_broadcast([128, C]),
                                                op=mybir.AluOpType.mult)
                        nc.vector.tensor_tensor(h0a[:, j, :], pt[:, :],
                                                am_sb[:, j, 0:1].to_broadcast([128, C]),
                                                op=mybir.AluOpType.mult)
                    nc.sync.dma_start(
                        bounce.ap()[n0:n0 + nn, 0:C].rearrange("(b p) c -> p b c", p=128),
                        hs_stage[:, 0:nb, :])

            # ---------------- propagation ----------------
            assert NG % 2 == 0
            HG = NG // 2
            with (
                tc.tile_pool(name="gat", bufs=4) as gp,
                tc.tile_pool(name="gixall", bufs=1) as ipall,
                tc.tile_pool(name="mt", bufs=2) as mtp,
                tc.tile_pool(name="hop", bufs=1) as hopp,
            ):
                gidx_sb = ipall.tile([128, meta.GW], I16)
                nc.sync.dma_start(gidx_sb[:, :], gidx_d.ap())
                qctr = 0
                h_sb = None
                for k in range(KHOPS):
                    hf = hfull[k % 2]
                    if not DEBUG_NOAG:
                        nc.gpsimd.collective_compute(
                            "AllGather", mybir.AluOpType.bypass, replica_groups=rg,
                            ins=[bounce.ap().opt()], outs=[hf.ap().opt()])

                    if DEBUG_LEVEL < 2:
                        continue
                    goff = 0
                    slice_chunks = {t: [ch for ch in meta.chunks if ch[0] == t]
                                    for t in range(NSLICES)}
                    msum = hopp.tile([128, NG, C], F32, tag="msum")
                    macc = hopp.tile([128, NG, C], F32, tag="macc")
                    for t in meta.slice_order:
                        # slice 0 reduces straight into msum (common order)
                        mt_sb = msum if t == 0 else mtp.tile([128, NG, C], F32,
                                                             tag="mt")
                        for (_, g0, g1, c0, cols) in slice_chunks[t]:
                            if cols > 0:
                                n_idx = 128 * cols
                                G = gp.tile([128, cfg.GCOLS_MAX, TW], F32, tag="G")
                                nc.gpsimd.dma_gather(
                                    G[:, 0:cols, :],
                                    hf.ap()[t * cfg.SLICE_ROWS:(t + 1) * cfg.SLICE_ROWS, :],
                                    gidx_sb[:, goff:goff + 8 * cols], n_idx, n_idx, TW,
                                    single_packet=False, queue_num=qctr % 4)
                                qctr += 1
                                goff += 8 * cols
                            if DEBUG_LEVEL < 3:
                                continue
                            g, cc = g0, 0
                            while g < g1:
                                d = int(meta.D[t, g])
                                ge = g
                                while ge < g1 and int(meta.D[t, ge]) == d:
                                    ge += 1
                                if d == 0:
                                    nc.vector.memset(mt_sb[:, g:ge, :], 0.0)
                                else:
                                    gv = G[:, cc:cc + (ge - g) * d, 0:C].rearrange(
                                        "p (g r) c -> p g c r", r=d)
                                    nc.vector.tensor_reduce(
                                        mt_sb[:, g:ge, :], gv,
                                        axis=mybir.AxisListType.X,
                                        op=mybir.AluOpType.add)
                                cc += (ge - g) * d
                                g = ge
                        if DEBUG_LEVEL >= 4 and t > 0:
                            nc.sync.dma_start(
                                Mt[t].ap()[:, 0:C].rearrange("(g p) c -> p g c", p=128),
                                mt_sb[:, :, :])

                    if DEBUG_LEVEL < 5:
                        continue
                    # combine slices 1..3 into msum via half-size permuted
                    # gathers sharing the G pool
                    for t in range(1, NSLICES):
                        for h in range(2):
                            cg = gp.tile([128, HG, TW], F32, tag="G")
                            nc.gpsimd.dma_gather(
                                cg[:, :, :], Mt[t].ap()[:, :],
                                cidx_sb[t][:, h * (HG * 8):(h + 1) * (HG * 8)],
                                128 * HG, 128 * HG, TW,
                                single_packet=False, queue_num=qctr % 4)
                            qctr += 1
                            nc.vector.tensor_add(
                                msum[:, h * HG:(h + 1) * HG, :],
                                msum[:, h * HG:(h + 1) * HG, :],
                                cg[:, :, 0:C])

                    if DEBUG_LEVEL < 6:
                        continue
                    nc.vector.tensor_tensor(msum[:, :, :], msum[:, :, :],
                                            nip_sb[:, :, 0:1].to_broadcast([128, NG, C]),
                                            op=mybir.AluOpType.mult)
                    h_sb = msum
                    nc.vector.tensor_add(h_sb[:, :, :], h_sb[:, :, :], h0a[:, :, :])
                    if k < KHOPS - 1:
                        hs_sb = hopp.tile([128, NG, C], F32, tag="hs")
                        nc.vector.tensor_tensor(hs_sb[:, :, :], h_sb[:, :, :],
                                                no_sb[:, :, 0:1].to_broadcast([128, NG, C]),
                                                op=mybir.AluOpType.mult)
                        nc.sync.dma_start(
                            bounce.ap()[:, 0:C].rearrange("(g p) c -> p g c", p=128),
                            hs_sb[:, :, :])

                if h_sb is None:
                    h_sb = h0a
                nc.sync.dma_start(
                    out_d.ap().rearrange("(g p) c -> p g c", p=128), h_sb[:, :, :])

    nc.compile()
    return nc


def run_kernel(meta, inputs, nc=None, trace=False):
    cfg = meta.cfg
    if nc is None:
        nc = build_nc(meta)
    features = np.asarray(inputs["features"], dtype=np.float32)
    in_maps = []
    for c in range(NCORES):
        xTa = np.zeros((FIN, cfg.PER), dtype=np.float32)
        xTa[:, :cfg.real_per] = features[meta.com_order[c]].T
        in_maps.append({
            "xT": np.ascontiguousarray(xTa),
            "W0": np.asarray(inputs["W0"], dtype=np.float32),
            "b0": np.asarray(inputs["b0"], dtype=np.float32),
            "W1": np.asarray(inputs["W1"], dtype=np.float32),
            "b1": np.asarray(inputs["b1"], dtype=np.float32),
            "W2": np.asarray(inputs["W2"], dtype=np.float32),
            "b2": np.asarray(inputs["b2"], dtype=np.float32),
            "no": meta.no_c[c],
            "nip": meta.nip_c[c],
            "am": meta.am_c[c],
            "gidx": meta.gidx[c],
            "ident": np.eye(128, dtype=np.float32),
            "cidx": meta.cidx[c],
            "out": np.zeros((cfg.PER, C), dtype=np.float32),
        })
    res = run_bass_kernel_spmd(nc, in_maps, core_ids=list(range(NCORES)),
                               trace=trace)
    out = np.zeros((cfg.N, C), dtype=np.float32)
    for c in range(NCORES):
        oc = np.asarray(res.results[c]["out"])
        out[meta.com_order[c]] = oc[:cfg.real_per]
    return out, res


def kernel(**inputs):
    cfg = Config(100000)
    meta = preprocess(np.asarray(inputs["edge_index"]), cfg)
    out, _ = run_kernel(meta, inputs)
    return out

